# revision 11
# baseline (speedup 1.0000x reference)
"""MLA (DeepSeek-style multi-head latent attention) kernel for Trainium2.

Problem: nn_MultiHeadAttention_28243704939173
  B=2, S=2048, D=2048, H=16, KV_RANK=512, NOPE=128, ROPE=64, V_HD=128.

Sharding (8 NeuronCores): DP=2 over batch x TP=4 over heads (4 heads per
core); the kv latent is computed replicated on every TP rank (as in real
MLA serving). Each core produces its heads' partial wo projection; the
host sums the 4 TP partials per batch element and adds wo_b.

Numerics: matmuls run in fp32r (fp32 with 11-bit mantissa; full PE rate)
accumulating into fp32 PSUM. Softmax skips the max-subtraction pass
(|scores * scale| < ~3 for this problem family so exp cannot overflow;
masked scores map to exp == 0 exactly). The per-(head, q-block) softmax
normalizer 1/Z folds into the PV-result copy (q on partitions there).
"""
import os
import numpy as np
from contextlib import ExitStack

import concourse.bass as bass
import concourse.bacc as bacc
import concourse.mybir as mybir
import concourse.tile as tile
from concourse import bass_utils

F32 = mybir.dt.float32
F32R = mybir.dt.float32r
AF = mybir.ActivationFunctionType
ALU = mybir.AluOpType
AX = mybir.AxisListType

B, S, D = 2, 2048, 2048
H = 16
KV = 512
NOPE, ROPE = 128, 64
QK_HD = NOPE + ROPE
V_HD = 128
SCALE = float(QK_HD) ** -0.5
EPS = 1.1920929e-07
NEG = -1.0e5  # mask addend; NEG*SCALE ~ -7220 -> exp underflows to exactly 0
HL = 4        # local heads per core (TP degree 4)
TP = 4
N_CORES = 8
KD = D // 128  # contraction chunks over the model dim


def round_f32r(a: np.ndarray) -> np.ndarray:
    """Round fp32 -> fp32r (11-bit mantissa, RNE), keeping fp32 container."""
    u = np.ascontiguousarray(a, dtype=np.float32).view(np.uint32).copy()
    lsb = (u >> np.uint32(12)) & np.uint32(1)
    u += np.uint32(0x7FF) + lsb
    u &= np.uint32(0xFFFFF000)
    return u.view(np.float32)


def build(s_len: int, q_bias: bool, kv_bias: bool, max_phase: int = 4):
    NB = s_len // 128
    NG = max(s_len // 512, 1)

    nc = bacc.Bacc("TRN2", target_bir_lowering=False, debug=False)

    xt = nc.dram_tensor("xt", [NB, 128, D], F32R, kind="ExternalInput")
    wq = nc.dram_tensor("wq", [128, KD * 768], F32R, kind="ExternalInput")
    wkv = nc.dram_tensor("wkv", [128, KD * 576], F32R, kind="ExternalInput")
    wbm = nc.dram_tensor("wbm", [128, HL * KV], F32R, kind="ExternalInput")
    wvt = nc.dram_tensor("wvt", [128, HL * 512], F32R, kind="ExternalInput")
    wot = nc.dram_tensor("wot", [128, HL * D], F32R, kind="ExternalInput")
    cosq = nc.dram_tensor("cosq", [128, NB * 64], F32, kind="ExternalInput")
    sinq = nc.dram_tensor("sinq", [128, NB * 64], F32, kind="ExternalInput")
    dmask = nc.dram_tensor("dmask", [128, 128], F32, kind="ExternalInput")
    identr = nc.dram_tensor("identr", [128, 128], F32R, kind="ExternalInput")
    identf = nc.dram_tensor("identf", [128, 128], F32, kind="ExternalInput")
    if q_bias:
        qb = nc.dram_tensor("qb", [1, 768], F32R, kind="ExternalInput")
    if kv_bias:
        kvb = nc.dram_tensor("kvb", [1, 576], F32R, kind="ExternalInput")
    out = nc.dram_tensor("out", [s_len, D], F32, kind="ExternalOutput")
    qnt_dram = nc.dram_tensor("qnt_dram", [HL, 128, s_len], F32R, kind="Internal")
    ot_dram = nc.dram_tensor("ot_dram", [HL, 128, s_len], F32R, kind="Internal")
    kpe_bnc = nc.dram_tensor("kpe_bnc", [64, s_len], F32R, kind="Internal")

    with tile.TileContext(nc) as tc, ExitStack() as ctx:
        # ---------------- pools/tensors that live across phases -------------
        persist = ctx.enter_context(tc.tile_pool(name="persist", bufs=1))
        qpepool = ctx.enter_context(tc.tile_pool(name="qpepool", bufs=2))

        kv_sb = persist.tile([128, NB * KV], F32R, tag="kv_sb")
        kvt_sb = persist.tile([128, 4 * s_len], F32R, tag="kvt_sb")
        # kpeT duplicated on both partition halves so either 64-base qpeT
        # slice can pair with a matching-base kpeT slice in the rope matmul
        kpet_sb = persist.tile([128, s_len], F32R, tag="kpet_sb")
        identr_sb = persist.tile([128, 128], F32R, tag="identr_sb")
        identf_sb = persist.tile([128, 128], F32, tag="identf_sb")
        dmask_sb = persist.tile([128, 128], F32, tag="dmask_sb")
        cosq_sb = persist.tile([128, NB * 64], F32, tag="cosq_sb")
        sinq_sb = persist.tile([128, NB * 64], F32, tag="sinq_sb")

        nc.sync.dma_start(identr_sb[:], identr.ap()[:])
        nc.sync.dma_start(identf_sb[:], identf.ap()[:])
        nc.sync.dma_start(dmask_sb[:], dmask.ap()[:])
        nc.sync.dma_start(cosq_sb[:], cosq.ap()[:])
        nc.sync.dma_start(sinq_sb[:], sinq.ap()[:])

        qpet = [qpepool.tile([128, s_len], F32R, tag="qpepool", name=f"qpet{pp}")
                for pp in range(2)]

        # ========== Phase 1: kv latent projection + rmsnorm + k rope ==========
        with tc.tile_pool(name="p1w", bufs=1) as p1w, \
                tc.tile_pool(name="p1", bufs=2) as p1, \
                tc.tile_pool(name="p1s", bufs=2) as p1s, \
                tc.tile_pool(name="ps1a", bufs=2, space="PSUM") as ps1a, \
                tc.tile_pool(name="ps1b", bufs=2, space="PSUM") as ps1b, \
                tc.tile_pool(name="ps1t", bufs=2, space="PSUM") as ps1t:
            wkv_sb = p1w.tile([128, KD * 576], F32R, tag="wkv_sb")
            nc.sync.dma_start(wkv_sb[:], wkv.ap()[:])
            if kv_bias:
                kvb_sb = p1w.tile([1, 576], F32R, tag="kvb_sb")
                nc.sync.dma_start(kvb_sb[:], kvb.ap()[:])
                ones1 = p1w.tile([1, 128], F32R, tag="ones1")
                nc.vector.memset(ones1[:], 1.0)
            for s in range(NB):
                xts = p1.tile([128, D], F32R, tag="xts")
                nc.sync.dma_start(xts[:], xt.ap()[s])
                pkv = ps1a.tile([128, 512], F32, tag="pkv")
                pkp = ps1b.tile([128, 64], F32, tag="pkp")
                for k in range(KD):
                    lhs = xts[:, 128 * k:128 * (k + 1)]
                    nc.tensor.matmul(pkv[:], lhs, wkv_sb[:, 576 * k:576 * k + 512],
                                     start=(k == 0),
                                     stop=(k == KD - 1 and not kv_bias))
                    nc.tensor.matmul(pkp[:], lhs,
                                     wkv_sb[:, 576 * k + 512:576 * (k + 1)],
                                     start=(k == 0),
                                     stop=(k == KD - 1 and not kv_bias))
                if kv_bias:
                    nc.tensor.matmul(pkv[:], ones1[:], kvb_sb[:, 0:512],
                                     start=False, stop=True)
                    nc.tensor.matmul(pkp[:], ones1[:], kvb_sb[:, 512:576],
                                     start=False, stop=True)
                # rmsnorm over the 512 latent channels
                kvtile = p1.tile([128, 512], F32, tag="kvtile")
                nc.vector.tensor_copy(kvtile[:], pkv[:])
                sq = p1.tile([128, 512], F32, tag="sq")
                msq = p1s.tile([128, 1], F32, tag="msq")
                nc.scalar.activation(sq[:], kvtile[:], AF.Square, bias=0.0,
                                     scale=1.0, accum_out=msq[:])
                ms2 = p1s.tile([128, 1], F32, tag="ms2")
                nc.vector.tensor_scalar(ms2[:], msq[:], 1.0 / KV, EPS, ALU.mult,
                                        ALU.add)
                srt = p1s.tile([128, 1], F32, tag="srt")
                nc.scalar.sqrt(srt[:], ms2[:])
                rrt = p1s.tile([128, 1], F32, tag="rrt")
                nc.vector.reciprocal(rrt[:], srt[:])
                nc.vector.tensor_scalar(kv_sb[:, KV * s:KV * (s + 1)], kvtile[:],
                                        rrt[:], None, ALU.mult)
                # k_pe rope (free-dim interleaved pairs)
                kpe = p1s.tile([128, 64], F32, tag="kpe")
                nc.vector.tensor_copy(kpe[:], pkp[:])
                ksw = p1s.tile([128, 64], F32, tag="ksw")
                k3 = kpe[:].rearrange("p (i two) -> p i two", two=2)
                w3 = ksw[:].rearrange("p (i two) -> p i two", two=2)
                nc.vector.tensor_copy(w3[:, :, 0:1], k3[:, :, 1:2])
                nc.vector.tensor_copy(w3[:, :, 1:2], k3[:, :, 0:1])
                krot = p1s.tile([128, 64], F32, tag="krot")
                nc.vector.tensor_mul(krot[:], kpe[:], cosq_sb[:, 64 * s:64 * (s + 1)])
                nc.vector.tensor_mul(ksw[:], ksw[:], sinq_sb[:, 64 * s:64 * (s + 1)])
                nc.vector.tensor_add(krot[:], krot[:], ksw[:])
                ptk = ps1t.tile([64, 128], F32, tag="ptk")
                nc.tensor.transpose(ptk[:], krot[:], identf_sb[:])
                nc.vector.tensor_copy(kpet_sb[0:64, 128 * s:128 * (s + 1)], ptk[:])
                # transpose normed kv block into kvT
                for cc in range(4):
                    ptc = ps1t.tile([128, 128], F32R, tag="ptc")
                    nc.tensor.transpose(
                        ptc[:], kv_sb[:, KV * s + 128 * cc:KV * s + 128 * (cc + 1)],
                        identr_sb[:])
                    nc.vector.tensor_copy(
                        kvt_sb[:, s_len * cc + 128 * s:s_len * cc + 128 * (s + 1)],
                        ptc[:])
            # duplicate kpeT into the upper partition half via a DRAM bounce
            # (a same-tensor SBUF->SBUF DMA deadlocks on HW)
            nc.sync.dma_start(kpe_bnc.ap()[:], kpet_sb[0:64, :])
            nc.sync.dma_start(kpet_sb[64:128, :], kpe_bnc.ap()[:])

        # ========== Phase 2: q projection + q rope + transposes ==========
        if max_phase >= 2:
          with tc.tile_pool(name="p2w", bufs=1) as p2w, \
                  tc.tile_pool(name="p2", bufs=2) as p2, \
                  tc.tile_pool(name="ps2", bufs=2, space="PSUM") as ps2, \
                  tc.tile_pool(name="ps2t", bufs=2, space="PSUM") as ps2t:
            wq_sb = p2w.tile([128, KD * 768], F32R, tag="wq_sb")
            nc.sync.dma_start(wq_sb[:], wq.ap()[:])
            if q_bias:
                qb_sb = p2w.tile([1, 768], F32R, tag="qb_sb")
                nc.sync.dma_start(qb_sb[:], qb.ap()[:])
                ones2 = p2w.tile([1, 128], F32R, tag="ones2")
                nc.vector.memset(ones2[:], 1.0)
            for s in range(NB):
                xts = p2.tile([128, D], F32R, tag="xts2")
                nc.sync.dma_start(xts[:], xt.ap()[s])
                pq = ps2.tile([128, 768], F32, tag="pq")
                for k in range(KD):
                    lhs = xts[:, 128 * k:128 * (k + 1)]
                    nc.tensor.matmul(pq[:, 0:512], lhs,
                                     wq_sb[:, 768 * k:768 * k + 512],
                                     start=(k == 0),
                                     stop=(k == KD - 1 and not q_bias))
                    nc.tensor.matmul(pq[:, 512:768], lhs,
                                     wq_sb[:, 768 * k + 512:768 * (k + 1)],
                                     start=(k == 0),
                                     stop=(k == KD - 1 and not q_bias))
                if q_bias:
                    nc.tensor.matmul(pq[:, 0:512], ones2[:], qb_sb[:, 0:512],
                                     start=False, stop=True)
                    nc.tensor.matmul(pq[:, 512:768], ones2[:], qb_sb[:, 512:768],
                                     start=False, stop=True)
                qsb = p2.tile([128, 768], F32, tag="qsb")
                nc.scalar.copy(qsb[:], pq[:])
                # rope on cols 512:768 (4 heads x 64 interleaved pairs)
                qsw = p2.tile([128, 256], F32, tag="qsw")
                a3 = qsb[:, 512:768].rearrange("p (i two) -> p i two", two=2)
                w3 = qsw[:].rearrange("p (i two) -> p i two", two=2)
                nc.vector.tensor_copy(w3[:, :, 0:1], a3[:, :, 1:2])
                nc.vector.tensor_copy(w3[:, :, 1:2], a3[:, :, 0:1])
                for hh in range(HL):
                    rsl = qsb[:, 512 + 64 * hh:512 + 64 * (hh + 1)]
                    ssl = qsw[:, 64 * hh:64 * (hh + 1)]
                    nc.vector.tensor_mul(rsl, rsl, cosq_sb[:, 64 * s:64 * (s + 1)])
                    nc.vector.tensor_mul(ssl, ssl, sinq_sb[:, 64 * s:64 * (s + 1)])
                    nc.vector.tensor_add(rsl, rsl, ssl)
                # transposes into qnT (via DRAM) and qpeT pair tensors
                for hh in range(HL):
                    pt2 = ps2t.tile([128, 128], F32, tag="pt2")
                    nc.tensor.transpose(pt2[:], qsb[:, 128 * hh:128 * (hh + 1)],
                                        identf_sb[:])
                    qnstg = p2.tile([128, 128], F32R, tag="qnstg")
                    nc.vector.tensor_copy(qnstg[:], pt2[:])
                    nc.sync.dma_start(qnt_dram.ap()[hh, :, 128 * s:128 * (s + 1)],
                                      qnstg[:])
                for pp in range(2):
                    pt2 = ps2t.tile([128, 128], F32, tag="pt2")
                    nc.tensor.transpose(pt2[:],
                                        qsb[:, 512 + 128 * pp:512 + 128 * (pp + 1)],
                                        identf_sb[:])
                    nc.vector.tensor_copy(qpet[pp][:, 128 * s:128 * (s + 1)], pt2[:])

        # ========== Phase 3: attention per local head ==========
        if max_phase >= 3:
          with tc.tile_pool(name="p3w", bufs=1) as p3w, \
                  tc.tile_pool(name="qatp", bufs=1) as qatp, \
                  tc.tile_pool(name="qntp", bufs=2) as qntp, \
                  tc.tile_pool(name="expp", bufs=6) as expp, \
                  tc.tile_pool(name="p3", bufs=3) as p3, \
                  tc.tile_pool(name="otstp", bufs=2) as otstp, \
                  tc.tile_pool(name="ps3s", bufs=3, space="PSUM") as ps3s, \
                  tc.tile_pool(name="ps3a", bufs=2, space="PSUM") as ps3a, \
                  tc.tile_pool(name="ps3t", bufs=2, space="PSUM") as ps3t:
            wb_sb = p3w.tile([128, HL * KV], F32R, tag="wb_sb")
            nc.sync.dma_start(wb_sb[:], wbm.ap()[:])
            wvt_sb = p3w.tile([128, HL * 512], F32R, tag="wvt_sb")
            nc.sync.dma_start(wvt_sb[:], wvt.ap()[:])
            for h in range(HL):
                # ---- absorb: qaT_h[c, q] = (qn_h @ Wb'_h)^T, cc-major ----
                qnts = qntp.tile([128, s_len], F32R, tag="qnts")
                nc.sync.dma_start(qnts[:], qnt_dram.ap()[h])
                qat = qatp.tile([128, 4 * s_len], F32R, tag="qat")
                gw0 = min(512, s_len)
                for cc in range(4):
                    for g in range(NG):
                        pa = ps3a.tile([128, 512], F32, tag="pacc")
                        nc.tensor.matmul(
                            pa[:, 0:gw0],
                            wb_sb[:, KV * h + 128 * cc:KV * h + 128 * (cc + 1)],
                            qnts[:, 512 * g:512 * g + gw0],
                            start=True, stop=True)
                        nc.vector.tensor_copy(
                            qat[:, s_len * cc + 512 * g:s_len * cc + 512 * g + gw0],
                            pa[:, 0:gw0])
                otst = None
                for i in range(NB):
                    nk = 128 * (i + 1)
                    nts = (nk + 511) // 512
                    if i % 4 == 0:
                        otst = otstp.tile([128, 2048], F32R, tag="otst")
                    # ---- scores for q-block i over all key slices ----
                    expsl_tiles = []
                    zp = p3.tile([128, 4], F32, tag="zp")
                    for ts in range(nts):
                        t0 = 512 * ts
                        tw = min(512, nk - t0)
                        pss = ps3s.tile([128, 512], F32, tag="pss")
                        for cc in range(4):
                            nc.tensor.matmul(
                                pss[:, 0:tw],
                                qat[:, s_len * cc + 128 * i:
                                    s_len * cc + 128 * (i + 1)],
                                kvt_sb[:, s_len * cc + t0:s_len * cc + t0 + tw],
                                start=(cc == 0), stop=False, skip_group_check=True)
                        nc.tensor.matmul(
                            pss[:, 0:tw],
                            qpet[h // 2][64 * (h % 2):64 * (h % 2) + 64,
                                         128 * i:128 * (i + 1)],
                            kpet_sb[64 * (h % 2):64 * (h % 2) + 64, t0:t0 + tw],
                            start=False, stop=True, skip_group_check=True)
                        if t0 + tw == nk:
                            nc.vector.tensor_add(pss[:, tw - 128:tw],
                                                 pss[:, tw - 128:tw], dmask_sb[:])
                        expsl = expp.tile([128, 512], F32R, tag="expsl")
                        nc.scalar.activation(expsl[:, 0:tw], pss[:, 0:tw], AF.Exp,
                                             bias=0.0, scale=SCALE,
                                             accum_out=zp[:, ts:ts + 1])
                        expsl_tiles.append(expsl)
                    # ---- 1/Z for this (head, q-block) ----
                    if nts > 1:
                        zs = p3.tile([128, 1], F32, tag="zs")
                        nc.vector.reduce_sum(zs[:], zp[:, 0:nts], axis=AX.X)
                    else:
                        zs = zp
                    rq = p3.tile([128, 1], F32, tag="rq")
                    nc.vector.reciprocal(rq[:], zs[:, 0:1])
                    # ---- PV: transpose P 128-blocks, accumulate over t ----
                    po = ps3a.tile([128, 512], F32, tag="pacc")
                    for j in range(i + 1):
                        ts_j, off = divmod(128 * j, 512)
                        pt3 = ps3t.tile([128, 128], F32R, tag="pt3")
                        nc.tensor.transpose(pt3[:],
                                            expsl_tiles[ts_j][:, off:off + 128],
                                            identr_sb[:])
                        ptile = p3.tile([128, 128], F32R, tag="ptile")
                        nc.vector.tensor_copy(ptile[:], pt3[:])
                        nc.tensor.matmul(po[:], ptile[:],
                                         kv_sb[:, KV * j:KV * (j + 1)],
                                         start=(j == 0), stop=(j == i),
                                         skip_group_check=True)
                    # ---- normalize by 1/Z on the PSUM->SBUF copy ----
                    ocp = p3.tile([128, 512], F32R, tag="ocp")
                    nc.vector.tensor_scalar(ocp[:], po[:], rq[:], None, ALU.mult)
                    # ---- transpose normalized PV into group staging ----
                    for cc in range(4):
                        pt3 = ps3t.tile([128, 128], F32R, tag="pt3")
                        nc.tensor.transpose(pt3[:], ocp[:, 128 * cc:128 * (cc + 1)],
                                            identr_sb[:])
                        nc.vector.tensor_copy(
                            otst[:, 512 * cc + 128 * (i % 4):
                                 512 * cc + 128 * (i % 4 + 1)],
                            pt3[:])
                    # ---- after each 4-block group: oT_h[d, q] over c-chunks ----
                    if i % 4 == 3 or i == NB - 1:
                        g = i // 4
                        gw = 128 * (i % 4 + 1)
                        pot = ps3a.tile([128, 512], F32, tag="pacc")
                        for cc in range(4):
                            nc.tensor.matmul(
                                pot[:, 0:gw],
                                wvt_sb[:, 512 * h + 128 * cc:
                                       512 * h + 128 * (cc + 1)],
                                otst[:, 512 * cc:512 * cc + gw],
                                start=(cc == 0), stop=(cc == 3))
                        otg = p3.tile([128, 512], F32R, tag="otg")
                        nc.vector.tensor_copy(otg[:, 0:gw], pot[:, 0:gw])
                        nc.sync.dma_start(
                            ot_dram.ap()[h, :, 512 * g:512 * g + gw], otg[:, 0:gw])

        # ========== Phase 4: wo projection ==========
        if max_phase >= 4:
          with tc.tile_pool(name="p4w", bufs=1) as p4w, \
                  tc.tile_pool(name="p4", bufs=3) as p4, \
                  tc.tile_pool(name="ps4", bufs=2, space="PSUM") as ps4:
            wot_sb = p4w.tile([128, HL * D], F32R, tag="wot_sb")
            nc.sync.dma_start(wot_sb[:], wot.ap()[:])
            for i in range(NB):
                otq = []
                for dc in range(HL):
                    otqt = p4.tile([128, 128], F32R, tag="otq", bufs=8,
                                   name=f"otq{dc}")
                    nc.sync.dma_start(otqt[:],
                                      ot_dram.ap()[dc, :, 128 * i:128 * (i + 1)])
                    otq.append(otqt)
                for n in range(D // 512):
                    pw = ps4.tile([128, 512], F32, tag="pw")
                    for dc in range(HL):
                        nc.tensor.matmul(
                            pw[:], otq[dc][:],
                            wot_sb[:, D * dc + 512 * n:D * dc + 512 * (n + 1)],
                            start=(dc == 0), stop=(dc == HL - 1))
                    osb = p4.tile([128, 512], F32, tag="osb")
                    nc.vector.tensor_copy(osb[:], pw[:])
                    nc.sync.dma_start(
                        out.ap()[128 * i:128 * (i + 1), 512 * n:512 * (n + 1)],
                        osb[:])

    nc.compile()
    return nc


def make_core_inputs(core, x, freqs, wq_w, wq_b, wkv_a_w, wkv_a_b, kv_norm_w,
                     wkv_b_w, wo_w, s_len):
    """Host-side shard + layout prep for one core."""
    b, g = core // TP, core % TP
    NB = s_len // 128
    heads = [TP * g + hh for hh in range(HL)]  # heads for TP rank g

    ins = {}
    # xt[s, p, 128k+c] = x[b, 128s+c, 128k+p]
    xb = np.ascontiguousarray(x[b, :s_len])                       # [S, D]
    xt = xb.reshape(NB, 128, KD, 128).transpose(0, 3, 2, 1)       # [s, p, k, c]
    ins["xt"] = round_f32r(np.ascontiguousarray(xt).reshape(NB, 128, D))

    # wq rows: 4x nope(128) then 4x rope(64) for local heads -> [768, D]
    wq3 = wq_w.reshape(H, QK_HD, D)
    rows = [wq3[hg, :NOPE] for hg in heads] + [wq3[hg, NOPE:] for hg in heads]
    wq_sel = np.concatenate(rows, axis=0)                         # [768, D]
    wqt = wq_sel.T.reshape(KD, 128, 768).transpose(1, 0, 2)       # [p, k, 768]
    ins["wq"] = round_f32r(np.ascontiguousarray(wqt).reshape(128, KD * 768))

    wkvt = wkv_a_w.T.reshape(KD, 128, 576).transpose(1, 0, 2)
    ins["wkv"] = round_f32r(np.ascontiguousarray(wkvt).reshape(128, KD * 576))

    wkv_b3 = wkv_b_w.reshape(H, NOPE + V_HD, KV)
    wb_cols = [wkv_b3[hg, :NOPE] * kv_norm_w[None, :] for hg in heads]
    ins["wbm"] = round_f32r(np.concatenate(wb_cols, axis=1))      # [128, 4*512]

    wvt_cols = []
    for hg in heads:
        wv = wkv_b3[hg, NOPE:] * kv_norm_w[None, :]               # [128(d), 512(c)]
        wvt_cols.append(wv.T.reshape(4, 128, 128).transpose(1, 0, 2).reshape(128, 512))
    ins["wvt"] = round_f32r(np.concatenate(wvt_cols, axis=1))     # [128, 4*512]

    wo_cols = np.concatenate([wo_w[:, hg * V_HD:(hg + 1) * V_HD] for hg in heads],
                             axis=1)                              # [D, 512]
    wot = wo_cols.T.reshape(HL, 128, D).transpose(1, 0, 2)        # [p, dc, D]
    ins["wot"] = round_f32r(np.ascontiguousarray(wot).reshape(128, HL * D))

    # rope tables in [s-block(128), 64] free-pair layout
    fr = freqs[:s_len]                                            # [S, 32]
    cos2 = np.repeat(np.cos(fr), 2, axis=1).astype(np.float32)    # [S, 64]
    sin1 = np.sin(fr)
    sin2 = np.empty((s_len, ROPE), np.float32)
    sin2[:, 0::2] = -sin1
    sin2[:, 1::2] = sin1
    ins["cosq"] = np.ascontiguousarray(
        cos2.reshape(NB, 128, 64).transpose(1, 0, 2).reshape(128, NB * 64))
    ins["sinq"] = np.ascontiguousarray(
        sin2.reshape(NB, 128, 64).transpose(1, 0, 2).reshape(128, NB * 64))

    ins["dmask"] = np.where(np.triu(np.ones((128, 128), bool), k=1),
                            np.float32(NEG), np.float32(0.0))
    eye = np.eye(128, dtype=np.float32)
    ins["identr"] = eye
    ins["identf"] = eye

    if np.any(wq_b != 0.0):
        rows_b = [wq_b.reshape(H, QK_HD)[hg, :NOPE] for hg in heads] + \
                 [wq_b.reshape(H, QK_HD)[hg, NOPE:] for hg in heads]
        ins["qb"] = round_f32r(np.concatenate(rows_b)[None, :])
    if np.any(wkv_a_b != 0.0):
        ins["kvb"] = round_f32r(wkv_a_b[None, :])
    return ins


_nc_cache = {}


def get_nc(s_len, q_bias, kv_bias):
    key = (s_len, q_bias, kv_bias)
    if key not in _nc_cache:
        _nc_cache[key] = build(s_len, q_bias, kv_bias)
    return _nc_cache[key]


def run_cores(inputs, s_len=S, trace=False):
    """Build per-core shards, run the SPMD kernel, return (out, results)."""
    x = np.asarray(inputs["x"], np.float32)
    freqs = np.asarray(inputs["freqs"], np.float32)
    wq_w = np.asarray(inputs["wq_w"], np.float32)
    wq_b = np.asarray(inputs["wq_b"], np.float32)
    wkv_a_w = np.asarray(inputs["wkv_a_w"], np.float32)
    wkv_a_b = np.asarray(inputs["wkv_a_b"], np.float32)
    kv_norm_w = np.asarray(inputs["kv_norm_w"], np.float32)
    wkv_b_w = np.asarray(inputs["wkv_b_w"], np.float32)
    wo_w = np.asarray(inputs["wo_w"], np.float32)
    wo_b = np.asarray(inputs["wo_b"], np.float32)

    q_bias = bool(np.any(wq_b != 0.0))
    kv_bias = bool(np.any(wkv_a_b != 0.0))
    nc = get_nc(s_len, q_bias, kv_bias)
    in_maps = [
        make_core_inputs(c, x, freqs, wq_w, wq_b, wkv_a_w, wkv_a_b, kv_norm_w,
                         wkv_b_w, wo_w, s_len)
        for c in range(N_CORES)
    ]
    res = bass_utils.run_bass_kernel_spmd(nc, in_maps, core_ids=list(range(N_CORES)),
                                          trace=trace)
    out = np.empty((B, s_len, D), np.float32)
    for b in range(B):
        p = [res.results[TP * b + g]["out"] for g in range(TP)]
        out[b] = (p[0] + p[1]) + (p[2] + p[3])
    out += wo_b[None, None, :]
    return out, res


def kernel(**inputs) -> np.ndarray:
    out, _ = run_cores(inputs, s_len=S, trace=False)
    return out
